# revision 26
# baseline (speedup 1.0000x reference)
"""Devign model (GGNN + conv readout) Trainium2 kernel.

Data-parallel over the batch dim: 64 graphs -> 8 NeuronCores x 8 graphs.
Feature-major layout ([feature, node] on SBUF partitions) everywhere, so no
transposes are needed. The GGNN scatter-add aggregation is a dense matmul
against per-graph adjacency-count matrices A^T[(type,src), dst] built on the
host; both the per-edge messages m and A^T are stored as fp8e4m3 and the
aggregation runs as DoubleRow fp8 matmuls (2x PE throughput, fp32 PSUM
accumulation; the counts are small integers, so A^T is exact). All other
matmuls (message linear, GRU gates, conv readout) stay bf16 to keep the
weights exact. A^T stays resident in SBUF (loaded once, fp8). A nonzero
b_lin is applied exactly via a host-precomputed per-graph constant
blc[f,d] = sum_t b_lin[t,f]*indeg_t[d] added during the aggregation evac
(zero-cost when b_lin == 0, as here). The per-graph loop is software-
pipelined: graph g+1's message matmuls are emitted between graph g's
aggregation and GRU so the PE never waits on the aggregation evac.
"""

import contextlib

import numpy as np
import ml_dtypes

import concourse.bass as bass
import concourse.bacc as bacc
import concourse.mybir as mybir
import concourse.tile as tile
from concourse.bass_utils import run_bass_kernel_spmd

bf16 = ml_dtypes.bfloat16
f8 = ml_dtypes.float8_e4m3
FP32 = mybir.dt.float32
BF16 = mybir.dt.bfloat16
FP8 = mybir.dt.float8e4
DR = mybir.MatmulPerfMode.DoubleRow

# Problem constants (hardcoded per the spec).
B, N, DIN, D, T, NUM_STEPS = 64, 510, 256, 256, 4, 6
NPAD = 512          # padded nodes per graph
GPC = 8             # graphs per core
N_CORES = 8
P = 128
TD = T * D
L1 = 508            # conv1 output length (510 - 3 + 1)
LP1 = 253           # after pool(3,2)
L2Y = 253           # conv2 (k=1) output length
L2Z = 252           # convc2 (k=2) output length
LF = 126            # after pool(2,2)

AF = mybir.ActivationFunctionType
ALU = mybir.AluOpType

_NC_CACHE = {}


def _build_nc(bench_loop=1, use_blc=False):
    nc = bacc.Bacc("TRN2", target_bir_lowering=False, debug=False,
                   num_devices=N_CORES)

    # ---- DRAM parameters (per-core shapes) ----
    hT0_d = nc.declare_dram_parameter("hT0", [P, 2, GPC * NPAD], BF16, isOutput=False)
    AT8_d = nc.declare_dram_parameter("AT8", [GPC, P, 8, 2, NPAD], FP8, isOutput=False)
    wcat_d = nc.declare_dram_parameter("Wcat", [P, 2, TD], BF16, isOutput=False)
    blc_d = (nc.declare_dram_parameter("BLC", [P, 2, GPC * NPAD], BF16, isOutput=False)
             if use_blc else None)
    wih_d = nc.declare_dram_parameter("WihT", [P, 2, 3 * D], BF16, isOutput=False)
    whh_d = nc.declare_dram_parameter("WhhT", [P, 2, 3 * D], BF16, isOutput=False)
    brz_d = nc.declare_dram_parameter("brz", [P, 4], FP32, isOutput=False)
    bihn_d = nc.declare_dram_parameter("bihn", [P, 2], FP32, isOutput=False)
    bhhn_d = nc.declare_dram_parameter("bhhn", [P, 2], FP32, isOutput=False)
    w1_d = nc.declare_dram_parameter("W1T", [P, 6, D], BF16, isOutput=False)
    w2_d = nc.declare_dram_parameter("W2T", [P, 2, D], BF16, isOutput=False)
    wc1_d = nc.declare_dram_parameter("Wc1T", [P, 12, 2 * D], BF16, isOutput=False)
    wc2_d = nc.declare_dram_parameter("Wc2T", [P, 8, 2 * D], BF16, isOutput=False)
    cb1_d = nc.declare_dram_parameter("cb1", [P, 2], FP32, isOutput=False)
    cb2_d = nc.declare_dram_parameter("cb2", [P, 2], FP32, isOutput=False)
    cc1_d = nc.declare_dram_parameter("cc1", [P, 4], FP32, isOutput=False)
    cc2_d = nc.declare_dram_parameter("cc2", [P, 4], FP32, isOutput=False)
    wy_d = nc.declare_dram_parameter("wyT", [P, 2, 1], BF16, isOutput=False)
    wz_d = nc.declare_dram_parameter("wzT", [P, 4, 1], BF16, isOutput=False)
    mlpb_d = nc.declare_dram_parameter("mlpb", [1, 2], FP32, isOutput=False)
    out_d = nc.declare_dram_parameter("out", [1, GPC], FP32, isOutput=True)

    with tile.TileContext(nc) as tc:
        with (
            tc.tile_pool(name="const", bufs=1) as cst,
            tc.tile_pool(name="state", bufs=1) as st,
            tc.tile_pool(name="mp", bufs=5) as mp,
            tc.tile_pool(name="agp", bufs=3) as agp,
            tc.tile_pool(name="rzp", bufs=6) as rzp,
            tc.tile_pool(name="gp", bufs=5) as gp,
            tc.tile_pool(name="cvp", bufs=3) as cvp,
            tc.tile_pool(name="zpp", bufs=5) as zpp,
            tc.tile_pool(name="psm", bufs=8, space="PSUM") as psm,
        ):
            # ---- load constants ----
            wcat = cst.tile([P, 2, TD], BF16)
            nc.sync.dma_start(wcat[:], wcat_d[:])
            wih = cst.tile([P, 2, 3 * D], BF16)
            nc.sync.dma_start(wih[:], wih_d[:])
            whh = cst.tile([P, 2, 3 * D], BF16)
            nc.sync.dma_start(whh[:], whh_d[:])
            brz = cst.tile([P, 4], FP32)
            nc.sync.dma_start(brz[:], brz_d[:])
            bihn = cst.tile([P, 2], FP32)
            nc.sync.dma_start(bihn[:], bihn_d[:])
            bhhn = cst.tile([P, 2], FP32)
            nc.sync.dma_start(bhhn[:], bhhn_d[:])
            w1 = cst.tile([P, 6, D], BF16)
            nc.sync.dma_start(w1[:], w1_d[:])
            w2 = cst.tile([P, 2, D], BF16)
            nc.sync.dma_start(w2[:], w2_d[:])
            wc1 = cst.tile([P, 12, 2 * D], BF16)
            nc.sync.dma_start(wc1[:], wc1_d[:])
            wc2 = cst.tile([P, 8, 2 * D], BF16)
            nc.sync.dma_start(wc2[:], wc2_d[:])
            cb1 = cst.tile([P, 2], FP32)
            nc.sync.dma_start(cb1[:], cb1_d[:])
            cb2 = cst.tile([P, 2], FP32)
            nc.sync.dma_start(cb2[:], cb2_d[:])
            cc1 = cst.tile([P, 4], FP32)
            nc.sync.dma_start(cc1[:], cc1_d[:])
            cc2 = cst.tile([P, 4], FP32)
            nc.sync.dma_start(cc2[:], cc2_d[:])
            wy = cst.tile([P, 2, 1], BF16)
            nc.sync.dma_start(wy[:], wy_d[:])
            wz = cst.tile([P, 4, 1], BF16)
            nc.sync.dma_start(wz[:], wz_d[:])
            mlpb = cst.tile([1, 2], FP32)
            nc.sync.dma_start(mlpb[:], mlpb_d[:])
            # read-only across bench iterations: adjacency, feats, bias-const
            at8s = []
            feats = []
            blcs = []
            for g in range(GPC):
                a8 = cst.tile([P, 8, 2, NPAD], FP8, tag=f"at8{g}", name=f"at8{g}")
                nc.scalar.dma_start(a8[:], AT8_d[g])
                at8s.append(a8)
                fg = cst.tile([P, 2, NPAD], BF16, tag=f"feat{g}", name=f"feat{g}")
                nc.sync.dma_start(fg[:], hT0_d[:, :, g * NPAD:(g + 1) * NPAD])
                feats.append(fg)
                if blc_d is not None:
                    bc = cst.tile([P, 2, NPAD], BF16, tag=f"blc{g}", name=f"blc{g}")
                    nc.sync.dma_start(bc[:], blc_d[:, :, g * NPAD:(g + 1) * NPAD])
                    blcs.append(bc)

            loop_cm = (
                tc.For_i(0, bench_loop, 1,
                         hint_engines=tuple(mybir.EngineType[e] for e in
                                            ("PE", "DVE", "Activation", "SP", "Pool")))
                if bench_loop > 1 else contextlib.nullcontext()
            )
            with loop_cm:
                _kernel_body(nc, tc, locals())

    nc.compile()
    return nc


def _kernel_body(nc, tc, env):
    class E:
        pass
    e = E()
    e.__dict__.update(env)
    _emit_body(nc, tc, e)


def _emit_body(nc, tc, e):
    (cst, st, mp, agp, rzp, gp, cvp, zpp, psm) = (
        e.cst, e.st, e.mp, e.agp, e.rzp, e.gp, e.cvp, e.zpp, e.psm)
    (wcat, wih, whh, brz, bihn, bhhn) = (
        e.wcat, e.wih, e.whh, e.brz, e.bihn, e.bhhn)
    blc_d = e.blc_d
    (w1, w2, wc1, wc2, cb1, cb2, cc1, cc2, wy, wz, mlpb) = (
        e.w1, e.w2, e.wc1, e.wc2, e.cb1, e.cb2, e.cc1, e.cc2, e.wy, e.wz, e.mlpb)
    (hT0_d, out_d, AT8_d) = (e.hT0_d, e.out_d, e.AT8_d)
    (w1_d, w2_d, wc1_d, wc2_d, cb1_d, cb2_d, cc1_d, cc2_d, wy_d, wz_d,
     mlpb_d) = (e.w1_d, e.w2_d, e.wc1_d, e.wc2_d, e.cb1_d, e.cb2_d,
                e.cc1_d, e.cc2_d, e.wy_d, e.wz_d, e.mlpb_d)

    at8s, feats, blcs = e.at8s, e.feats, e.blcs
    # ---- per-graph h state tiles (feature-major [feat_part, kt, node]) ----
    hA, hB = [], []
    for g in range(GPC):
        h0 = st.tile([P, 2, NPAD], BF16, tag=f"hA{g}")
        nc.sync.dma_start(h0[:], hT0_d[:, :, g * NPAD:(g + 1) * NPAD])
        hA.append(h0)
        h1 = st.tile([P, 2, NPAD], BF16, tag=f"hB{g}", name=f"hB{g}")
        hB.append(h1)

    # ================= GGNN steps =================
    def emit_m(hg):
        """m = h @ Wcat (bf16 matmuls; fp8 node-pair-major evac)."""
        m8s = []
        for j in range(2):
            m8t = mp.tile([P, 2, TD], FP8, tag="m8")
            for ko in range(2):
                i = 2 * j + ko
                for nt in range(2):
                    pm = psm.tile([P, 512], FP32, tag="ps")
                    for kt in range(2):
                        nc.tensor.matmul(
                            pm[:],
                            lhsT=hg[:, kt, i * P:(i + 1) * P],
                            rhs=wcat[:, kt, nt * 512:(nt + 1) * 512],
                            start=(kt == 0), stop=(kt == 1),
                        )
                    dst = m8t[:, ko, nt * 512:(nt + 1) * 512]
                    if (i + nt) % 2 == 0:
                        nc.vector.tensor_copy(dst, pm[:])
                    else:
                        nc.scalar.activation(dst, pm[:], AF.Identity)
            m8s.append(m8t)
        return m8s

    for step in range(NUM_STEPS):
        hcur = hA if step % 2 == 0 else hB
        hnxt = hB if step % 2 == 0 else hA
        m8_all = {0: emit_m(hcur[0])}
        for g in range(GPC):
            hg = hcur[g]
            # --- aggregation: a^T = m_stack^T @ A^T_g (DoubleRow fp8) ---
            m8s = m8_all.pop(g)
            pas = [psm.tile([P, 512], FP32, tag="ps", name=f"pa{mt}")
                   for mt in range(2)]
            for c in range(8):
                j, t = c // 4, c % 4
                for mt in range(2):
                    nc.tensor.matmul(
                        pas[mt][:],
                        lhsT=m8s[j][:, :, t * D + mt * P: t * D + (mt + 1) * P],
                        rhs=at8s[g][:, 2 * t + j, :, :],
                        start=(c == 0), stop=(c == 7), perf_mode=DR,
                    )
            # overlap next graph's m-compute with this graph's GRU chain
            if g + 1 < GPC:
                m8_all[g + 1] = emit_m(hcur[g + 1])
            ag = agp.tile([P, 2, NPAD], BF16, tag="ag")
            if blcs:
                nc.vector.tensor_tensor(ag[:, 0, :], pas[0][:],
                                        blcs[g][:, 0, :], op=ALU.add)
                nc.vector.tensor_tensor(ag[:, 1, :], pas[1][:],
                                        blcs[g][:, 1, :], op=ALU.add)
            else:
                nc.vector.tensor_copy(ag[:, 0, :], pas[0][:])
                nc.scalar.activation(ag[:, 1, :], pas[1][:], AF.Identity)

            # --- GRU (bf16) ---
            rts = []
            for mt in range(4):
                pr = psm.tile([P, 512], FP32, tag="ps")
                for kt in range(2):
                    nc.tensor.matmul(
                        pr[:], lhsT=wih[:, kt, mt * P:(mt + 1) * P],
                        rhs=ag[:, kt, :], start=(kt == 0), stop=False)
                for kt in range(2):
                    nc.tensor.matmul(
                        pr[:], lhsT=whh[:, kt, mt * P:(mt + 1) * P],
                        rhs=hg[:, kt, :], start=False, stop=(kt == 1))
                rzt = rzp.tile([P, 512], BF16, tag="rz")
                nc.scalar.activation(rzt[:], pr[:], AF.Sigmoid,
                                     bias=brz[:, mt:mt + 1])
                rts.append(rzt)
            for mt in range(2):
                pi = psm.tile([P, 512], FP32, tag="ps")
                for kt in range(2):
                    nc.tensor.matmul(
                        pi[:], lhsT=wih[:, kt, 2 * D + mt * P: 2 * D + (mt + 1) * P],
                        rhs=ag[:, kt, :], start=(kt == 0), stop=(kt == 1))
                ph = psm.tile([P, 512], FP32, tag="ps")
                for kt in range(2):
                    nc.tensor.matmul(
                        ph[:], lhsT=whh[:, kt, 2 * D + mt * P: 2 * D + (mt + 1) * P],
                        rhs=hg[:, kt, :], start=(kt == 0), stop=(kt == 1))
                t1 = gp.tile([P, 512], BF16, tag="t1")
                nc.vector.scalar_tensor_tensor(
                    t1[:], ph[:], bhhn[:, mt:mt + 1], rts[mt][:],
                    op0=ALU.add, op1=ALU.mult)
                nc.vector.scalar_tensor_tensor(
                    t1[:], pi[:], bihn[:, mt:mt + 1], t1[:],
                    op0=ALU.add, op1=ALU.add)
                nsb = gp.tile([P, 512], BF16, tag="nsb")
                nc.scalar.activation(nsb[:], t1[:], AF.Tanh)
                dsb = gp.tile([P, 512], BF16, tag="dsb")
                nc.gpsimd.tensor_tensor(dsb[:], hg[:, mt, :], nsb[:],
                                        op=ALU.subtract)
                nc.gpsimd.tensor_tensor(dsb[:], rts[2 + mt][:], dsb[:],
                                        op=ALU.mult)
                nc.gpsimd.tensor_tensor(hnxt[g][:, mt, :], nsb[:], dsb[:],
                                        op=ALU.add)

    hfin = hA if NUM_STEPS % 2 == 0 else hB

    # ================= conv readout (bf16) =================
    res = cst.tile([1, GPC], FP32)
    for g in range(GPC):
        hg = hfin[g]
        fg = feats[g]
        # --- Y branch ---
        y1p = []
        for mt in range(2):
            pm = psm.tile([P, 512], FP32, tag="ps")
            first = True
            for k in range(3):
                for kt in range(2):
                    nc.tensor.matmul(
                        pm[:, :L1],
                        lhsT=w1[:, k * 2 + kt, mt * P:(mt + 1) * P],
                        rhs=hg[:, kt, k:k + L1],
                        start=first, stop=(k == 2 and kt == 1))
                    first = False
            y1 = cvp.tile([P, 512], BF16, tag="y1")
            if mt % 2 == 0:
                nc.vector.tensor_scalar(y1[:, :L1], pm[:, :L1],
                                        cb1[:, mt:mt + 1], 0.0,
                                        op0=ALU.add, op1=ALU.max)
            else:
                nc.scalar.activation(y1[:, :L1], pm[:, :L1], AF.Relu,
                                     bias=cb1[:, mt:mt + 1])
            yp = cvp.tile([P, LP1], BF16, tag="y1p")
            nc.vector.tensor_tensor(yp[:], y1[:, 0:505:2], y1[:, 1:506:2],
                                    op=ALU.max)
            nc.vector.tensor_tensor(yp[:], yp[:], y1[:, 2:507:2],
                                    op=ALU.max)
            y1p.append(yp)
        y2p = []
        for mt in range(2):
            pm = psm.tile([P, 512], FP32, tag="ps")
            for kt in range(2):
                nc.tensor.matmul(
                    pm[:, :L2Y],
                    lhsT=w2[:, kt, mt * P:(mt + 1) * P],
                    rhs=y1p[kt][:],
                    start=(kt == 0), stop=(kt == 1))
            y2 = cvp.tile([P, L2Y], BF16, tag="y2")
            if mt % 2 == 0:
                nc.vector.tensor_scalar(y2[:], pm[:, :L2Y],
                                        cb2[:, mt:mt + 1], 0.0,
                                        op0=ALU.add, op1=ALU.max)
            else:
                nc.scalar.activation(y2[:], pm[:, :L2Y], AF.Relu,
                                     bias=cb2[:, mt:mt + 1])
            yp = cvp.tile([P, LF], BF16, tag="y2p")
            nc.vector.tensor_tensor(yp[:], y2[:, 0:251:2], y2[:, 1:252:2],
                                    op=ALU.max)
            y2p.append(yp)
        pv = psm.tile([P, 512], FP32, tag="ps")
        for kt in range(2):
            nc.tensor.matmul(pv[0:1, :LF], lhsT=wy[:, kt, :],
                             rhs=y2p[kt][:], start=(kt == 0), stop=(kt == 1))
        ysb = cvp.tile([1, LF], FP32, tag="ysb")
        nc.scalar.activation(ysb[:], pv[0:1, :LF], AF.Identity,
                             bias=mlpb[:, 0:1])

        # --- Z branch (channels = [h; feat]) ---
        z1p = []
        for mt in range(4):
            pm = psm.tile([P, 512], FP32, tag="ps")
            first = True
            for k in range(3):
                for kt in range(4):
                    src = hg if kt < 2 else fg
                    nc.tensor.matmul(
                        pm[:, :L1],
                        lhsT=wc1[:, k * 4 + kt, mt * P:(mt + 1) * P],
                        rhs=src[:, kt % 2, k:k + L1],
                        start=first, stop=(k == 2 and kt == 3))
                    first = False
            z1 = cvp.tile([P, 512], BF16, tag="z1")
            if mt % 2 == 0:
                nc.vector.tensor_scalar(z1[:, :L1], pm[:, :L1],
                                        cc1[:, mt:mt + 1], 0.0,
                                        op0=ALU.add, op1=ALU.max)
            else:
                nc.scalar.activation(z1[:, :L1], pm[:, :L1], AF.Relu,
                                     bias=cc1[:, mt:mt + 1])
            zp = zpp.tile([P, LP1], BF16, tag="z1p")
            nc.vector.tensor_tensor(zp[:], z1[:, 0:505:2], z1[:, 1:506:2],
                                    op=ALU.max)
            nc.vector.tensor_tensor(zp[:], zp[:], z1[:, 2:507:2],
                                    op=ALU.max)
            z1p.append(zp)
        z2p = []
        for mt in range(4):
            pm = psm.tile([P, 512], FP32, tag="ps")
            first = True
            for k in range(2):
                for kt in range(4):
                    nc.tensor.matmul(
                        pm[:, :L2Z],
                        lhsT=wc2[:, k * 4 + kt, mt * P:(mt + 1) * P],
                        rhs=z1p[kt][:, k:k + L2Z],
                        start=first, stop=(k == 1 and kt == 3))
                    first = False
            z2 = cvp.tile([P, L2Z], BF16, tag="z2")
            if mt % 2 == 0:
                nc.vector.tensor_scalar(z2[:], pm[:, :L2Z],
                                        cc2[:, mt:mt + 1], 0.0,
                                        op0=ALU.add, op1=ALU.max)
            else:
                nc.scalar.activation(z2[:], pm[:, :L2Z], AF.Relu,
                                     bias=cc2[:, mt:mt + 1])
            zp = zpp.tile([P, LF], BF16, tag="z2p")
            nc.vector.tensor_tensor(zp[:], z2[:, 0:251:2], z2[:, 1:252:2],
                                    op=ALU.max)
            z2p.append(zp)
        pv = psm.tile([P, 512], FP32, tag="ps")
        for kt in range(4):
            nc.tensor.matmul(pv[0:1, :LF], lhsT=wz[:, kt, :],
                             rhs=z2p[kt][:], start=(kt == 0), stop=(kt == 3))
        zsb = cvp.tile([1, LF], FP32, tag="zsb")
        nc.scalar.activation(zsb[:], pv[0:1, :LF], AF.Identity,
                             bias=mlpb[:, 1:2])

        # --- final: sigmoid(mean(y*z)) ---
        prod = cvp.tile([1, LF], FP32, tag="prod")
        nc.vector.tensor_mul(prod[:], ysb[:], zsb[:])
        ssum = cvp.tile([1, 1], FP32, tag="ssum")
        nc.vector.reduce_sum(ssum[:], prod[:], axis=mybir.AxisListType.X)
        nc.scalar.activation(res[:, g:g + 1], ssum[:], AF.Sigmoid,
                             scale=1.0 / LF)

    for g in range(GPC):
        emit_conv(g)

    nc.sync.dma_start(out_d[:], res[:])


def _split_part(a, ntile):
    """[ntile*128, F...] -> [128, ntile, F...] with [p, t, ...] = a[t*128+p, ...]."""
    return np.ascontiguousarray(
        a.reshape(ntile, P, *a.shape[1:]).transpose(1, 0, *range(2, a.ndim + 1)))


def _prep_inputs(inputs):
    feat = np.asarray(inputs["feat"], np.float32)
    esrc = np.asarray(inputs["edge_src"]).astype(np.int64)
    edst = np.asarray(inputs["edge_dst"]).astype(np.int64)
    etyp = np.asarray(inputs["edge_type"]).astype(np.int64)

    # feature-major padded feat^T: per graph [256, 512]
    ftp = np.zeros((B, DIN, NPAD), np.float32)
    ftp[:, :, :N] = feat.transpose(0, 2, 1)

    # adjacency: AT[g, ki, t, j, ko, d] = #edges(type=t, src=256j+128ko+ki, dst=d)
    AT_all = np.zeros((B, P, T, 2, 2, NPAD), np.float32)
    DT_all = np.zeros((B, T, NPAD), np.float32)
    g_of = esrc // N
    s_l = esrc % N
    d_l = edst % N
    np.add.at(AT_all, (g_of, s_l % P, etyp, s_l // 256, (s_l % 256) // P, d_l), 1.0)
    np.add.at(DT_all, (g_of, etyp, d_l), 1.0)
    AT_all = AT_all.reshape(B, P, 8, 2, NPAD)
    b_lin = np.asarray(inputs["b_lin"], np.float32)
    use_blc = bool(np.any(b_lin))

    W_lin = np.asarray(inputs["W_lin"], np.float32)
    Wcat = W_lin.transpose(2, 0, 1).reshape(D, TD)
    W_ih = np.asarray(inputs["W_ih"], np.float32)
    W_hh = np.asarray(inputs["W_hh"], np.float32)
    b_ih = np.asarray(inputs["b_ih"], np.float32)
    b_hh = np.asarray(inputs["b_hh"], np.float32)

    def convT(w):  # [O, I, K] -> [128, K*ktiles, O]
        O, I, K = w.shape
        kt = I // P
        arr = w.transpose(2, 1, 0).reshape(K, kt, P, O).transpose(2, 0, 1, 3)
        return np.ascontiguousarray(arr.reshape(P, K * kt, O))

    common = {
        "Wcat": _split_part(Wcat, 2).astype(bf16),
        "WihT": _split_part(np.ascontiguousarray(W_ih.T), 2).astype(bf16),
        "WhhT": _split_part(np.ascontiguousarray(W_hh.T), 2).astype(bf16),
        "brz": np.ascontiguousarray((b_ih + b_hh)[:2 * D].reshape(4, P).T),
        "bihn": np.ascontiguousarray(b_ih[2 * D:].reshape(2, P).T),
        "bhhn": np.ascontiguousarray(b_hh[2 * D:].reshape(2, P).T),
        "W1T": convT(np.asarray(inputs["conv1_w"], np.float32)).astype(bf16),
        "W2T": convT(np.asarray(inputs["conv2_w"], np.float32)).astype(bf16),
        "Wc1T": convT(np.asarray(inputs["convc1_w"], np.float32)).astype(bf16),
        "Wc2T": convT(np.asarray(inputs["convc2_w"], np.float32)).astype(bf16),
        "cb1": np.ascontiguousarray(np.asarray(inputs["conv1_b"], np.float32).reshape(2, P).T),
        "cb2": np.ascontiguousarray(np.asarray(inputs["conv2_b"], np.float32).reshape(2, P).T),
        "cc1": np.ascontiguousarray(np.asarray(inputs["convc1_b"], np.float32).reshape(4, P).T),
        "cc2": np.ascontiguousarray(np.asarray(inputs["convc2_b"], np.float32).reshape(4, P).T),
        "wyT": _split_part(np.ascontiguousarray(np.asarray(inputs["mlp_y_w"], np.float32).T), 2).astype(bf16),
        "wzT": _split_part(np.ascontiguousarray(np.asarray(inputs["mlp_z_w"], np.float32).T), 4).astype(bf16),
        "mlpb": np.array([[float(np.asarray(inputs["mlp_y_b"])[0]),
                           float(np.asarray(inputs["mlp_z_b"])[0])]], np.float32),
    }

    in_maps = []
    for c in range(N_CORES):
        sl = slice(c * GPC, (c + 1) * GPC)
        hT0 = ftp[sl].transpose(1, 0, 2).reshape(DIN, GPC * NPAD)
        m = dict(common)
        m["hT0"] = _split_part(hT0, 2).astype(bf16)
        m["AT8"] = AT_all[sl].astype(f8)
        if use_blc:
            # blc[f, d] = sum_t b_lin[t, f] * indeg_t[d], per graph
            blc = np.einsum("tf,gtd->gfd", b_lin, DT_all[sl])
            blc = blc.transpose(1, 0, 2).reshape(D, GPC * NPAD)
            m["BLC"] = _split_part(blc, 2).astype(bf16)
        in_maps.append(m)
    return in_maps


def kernel(**inputs):
    use_blc = bool(np.any(np.asarray(inputs["b_lin"])))
    key = ("nc", use_blc)
    if key not in _NC_CACHE:
        _NC_CACHE[key] = _build_nc(use_blc=use_blc)
    nc = _NC_CACHE[key]
    in_maps = _prep_inputs(inputs)
    res = run_bass_kernel_spmd(nc, in_maps, list(range(N_CORES)))
    return np.concatenate([res.results[c]["out"][0] for c in range(N_CORES)])


# revision 29
# speedup vs baseline: 1.0238x; 1.0238x over previous
"""Devign model (GGNN + conv readout) Trainium2 kernel.

Data-parallel over the batch dim: 64 graphs -> 8 NeuronCores x 8 graphs.
Feature-major layout ([feature, node] on SBUF partitions) everywhere, so no
transposes are needed. The GGNN scatter-add aggregation is a dense matmul
against per-graph adjacency-count matrices A^T[(type,src), dst] built on the
host; both the per-edge messages m and A^T are stored as fp8e4m3 and the
aggregation runs as DoubleRow fp8 matmuls (2x PE throughput, fp32 PSUM
accumulation; the counts are small integers, so A^T is exact). All other
matmuls (message linear, GRU gates, conv readout) stay bf16 to keep the
weights exact. A^T stays resident in SBUF (loaded once, fp8). A nonzero
b_lin is applied exactly via a host-precomputed per-graph constant
blc[f,d] = sum_t b_lin[t,f]*indeg_t[d] added during the aggregation evac
(zero-cost when b_lin == 0, as here). The per-graph loop is software-
pipelined: graph g+1's message matmuls are emitted between graph g's
aggregation and GRU so the PE never waits on the aggregation evac.
"""

import contextlib

import numpy as np
import ml_dtypes

import concourse.bass as bass
import concourse.bacc as bacc
import concourse.mybir as mybir
import concourse.tile as tile
from concourse.bass_utils import run_bass_kernel_spmd

bf16 = ml_dtypes.bfloat16
f8 = ml_dtypes.float8_e4m3
FP32 = mybir.dt.float32
BF16 = mybir.dt.bfloat16
FP8 = mybir.dt.float8e4
DR = mybir.MatmulPerfMode.DoubleRow

# Problem constants (hardcoded per the spec).
B, N, DIN, D, T, NUM_STEPS = 64, 510, 256, 256, 4, 6
NPAD = 512          # padded nodes per graph
GPC = 8             # graphs per core
N_CORES = 8
P = 128
TD = T * D
L1 = 508            # conv1 output length (510 - 3 + 1)
LP1 = 253           # after pool(3,2)
L2Y = 253           # conv2 (k=1) output length
L2Z = 252           # convc2 (k=2) output length
LF = 126            # after pool(2,2)

AF = mybir.ActivationFunctionType
ALU = mybir.AluOpType

_NC_CACHE = {}


def _build_nc(bench_loop=1, use_blc=False):
    nc = bacc.Bacc("TRN2", target_bir_lowering=False, debug=False,
                   num_devices=N_CORES)

    # ---- DRAM parameters (per-core shapes) ----
    hT0_d = nc.declare_dram_parameter("hT0", [P, 2, GPC * NPAD], BF16, isOutput=False)
    AT8_d = nc.declare_dram_parameter("AT8", [GPC, P, 8, 2, NPAD], FP8, isOutput=False)
    wcat_d = nc.declare_dram_parameter("Wcat", [P, 2, TD], BF16, isOutput=False)
    blc_d = (nc.declare_dram_parameter("BLC", [P, 2, GPC * NPAD], BF16, isOutput=False)
             if use_blc else None)
    wih_d = nc.declare_dram_parameter("WihT", [P, 2, 3 * D], BF16, isOutput=False)
    whh_d = nc.declare_dram_parameter("WhhT", [P, 2, 3 * D], BF16, isOutput=False)
    brz_d = nc.declare_dram_parameter("brz", [P, 4], FP32, isOutput=False)
    bihn_d = nc.declare_dram_parameter("bihn", [P, 2], FP32, isOutput=False)
    bhhn_d = nc.declare_dram_parameter("bhhn", [P, 2], FP32, isOutput=False)
    w1_d = nc.declare_dram_parameter("W1T", [P, 6, D], BF16, isOutput=False)
    w2_d = nc.declare_dram_parameter("W2T", [P, 2, D], BF16, isOutput=False)
    wc1_d = nc.declare_dram_parameter("Wc1T", [P, 12, 2 * D], BF16, isOutput=False)
    wc2_d = nc.declare_dram_parameter("Wc2T", [P, 8, 2 * D], BF16, isOutput=False)
    cb1_d = nc.declare_dram_parameter("cb1", [P, 2], FP32, isOutput=False)
    cb2_d = nc.declare_dram_parameter("cb2", [P, 2], FP32, isOutput=False)
    cc1_d = nc.declare_dram_parameter("cc1", [P, 4], FP32, isOutput=False)
    cc2_d = nc.declare_dram_parameter("cc2", [P, 4], FP32, isOutput=False)
    wy_d = nc.declare_dram_parameter("wyT", [P, 2, 1], BF16, isOutput=False)
    wz_d = nc.declare_dram_parameter("wzT", [P, 4, 1], BF16, isOutput=False)
    mlpb_d = nc.declare_dram_parameter("mlpb", [1, 2], FP32, isOutput=False)
    out_d = nc.declare_dram_parameter("out", [1, GPC], FP32, isOutput=True)

    with tile.TileContext(nc) as tc:
        with (
            tc.tile_pool(name="const", bufs=1) as cst,
            tc.tile_pool(name="state", bufs=1) as st,
            tc.tile_pool(name="mp", bufs=4) as mp,
            tc.tile_pool(name="agp", bufs=2) as agp,
            tc.tile_pool(name="rzp", bufs=3) as rzp,
            tc.tile_pool(name="gp", bufs=4) as gp,
            tc.tile_pool(name="cvp", bufs=3) as cvp,
            tc.tile_pool(name="zpp", bufs=4) as zpp,
            tc.tile_pool(name="psm", bufs=8, space="PSUM") as psm,
        ):
            # ---- load constants ----
            wcat = cst.tile([P, 2, TD], BF16)
            nc.sync.dma_start(wcat[:], wcat_d[:])
            wih = cst.tile([P, 2, 3 * D], BF16)
            nc.sync.dma_start(wih[:], wih_d[:])
            whh = cst.tile([P, 2, 3 * D], BF16)
            nc.sync.dma_start(whh[:], whh_d[:])
            brz = cst.tile([P, 4], FP32)
            nc.sync.dma_start(brz[:], brz_d[:])
            bihn = cst.tile([P, 2], FP32)
            nc.sync.dma_start(bihn[:], bihn_d[:])
            bhhn = cst.tile([P, 2], FP32)
            nc.sync.dma_start(bhhn[:], bhhn_d[:])
            w1 = cst.tile([P, 6, D], BF16)
            nc.sync.dma_start(w1[:], w1_d[:])
            w2 = cst.tile([P, 2, D], BF16)
            nc.sync.dma_start(w2[:], w2_d[:])
            wc1 = cst.tile([P, 12, 2 * D], BF16)
            nc.sync.dma_start(wc1[:], wc1_d[:])
            wc2 = cst.tile([P, 8, 2 * D], BF16)
            nc.sync.dma_start(wc2[:], wc2_d[:])
            cb1 = cst.tile([P, 2], FP32)
            nc.sync.dma_start(cb1[:], cb1_d[:])
            cb2 = cst.tile([P, 2], FP32)
            nc.sync.dma_start(cb2[:], cb2_d[:])
            cc1 = cst.tile([P, 4], FP32)
            nc.sync.dma_start(cc1[:], cc1_d[:])
            cc2 = cst.tile([P, 4], FP32)
            nc.sync.dma_start(cc2[:], cc2_d[:])
            wy = cst.tile([P, 2, 1], BF16)
            nc.sync.dma_start(wy[:], wy_d[:])
            wz = cst.tile([P, 4, 1], BF16)
            nc.sync.dma_start(wz[:], wz_d[:])
            mlpb = cst.tile([1, 2], FP32)
            nc.sync.dma_start(mlpb[:], mlpb_d[:])
            # read-only across bench iterations: adjacency, feats, bias-const
            at8s = []
            feats = []
            blcs = []
            for g in range(GPC):
                a8 = cst.tile([P, 8, 2, NPAD], FP8, tag=f"at8{g}", name=f"at8{g}")
                nc.scalar.dma_start(a8[:], AT8_d[g])
                at8s.append(a8)
                fg = cst.tile([P, 2, NPAD], BF16, tag=f"feat{g}", name=f"feat{g}")
                nc.sync.dma_start(fg[:], hT0_d[:, :, g * NPAD:(g + 1) * NPAD])
                feats.append(fg)
                if blc_d is not None:
                    bc = cst.tile([P, 2, NPAD], BF16, tag=f"blc{g}", name=f"blc{g}")
                    nc.sync.dma_start(bc[:], blc_d[:, :, g * NPAD:(g + 1) * NPAD])
                    blcs.append(bc)

            loop_cm = (
                tc.For_i(0, bench_loop, 1,
                         hint_engines=tuple(mybir.EngineType[e] for e in
                                            ("PE", "DVE", "Activation", "SP", "Pool")))
                if bench_loop > 1 else contextlib.nullcontext()
            )
            with loop_cm:
                _kernel_body(nc, tc, locals())

    nc.compile()
    return nc


def _kernel_body(nc, tc, env):
    class E:
        pass
    e = E()
    e.__dict__.update(env)
    _emit_body(nc, tc, e)


def _emit_body(nc, tc, e):
    (cst, st, mp, agp, rzp, gp, cvp, zpp, psm) = (
        e.cst, e.st, e.mp, e.agp, e.rzp, e.gp, e.cvp, e.zpp, e.psm)
    (wcat, wih, whh, brz, bihn, bhhn) = (
        e.wcat, e.wih, e.whh, e.brz, e.bihn, e.bhhn)
    blc_d = e.blc_d
    (w1, w2, wc1, wc2, cb1, cb2, cc1, cc2, wy, wz, mlpb) = (
        e.w1, e.w2, e.wc1, e.wc2, e.cb1, e.cb2, e.cc1, e.cc2, e.wy, e.wz, e.mlpb)
    (hT0_d, out_d, AT8_d) = (e.hT0_d, e.out_d, e.AT8_d)
    (w1_d, w2_d, wc1_d, wc2_d, cb1_d, cb2_d, cc1_d, cc2_d, wy_d, wz_d,
     mlpb_d) = (e.w1_d, e.w2_d, e.wc1_d, e.wc2_d, e.cb1_d, e.cb2_d,
                e.cc1_d, e.cc2_d, e.wy_d, e.wz_d, e.mlpb_d)

    at8s, feats, blcs = e.at8s, e.feats, e.blcs
    # ---- per-graph h state tiles (feature-major [feat_part, kt, node]) ----
    hA, hB = [], []
    for g in range(GPC):
        h0 = st.tile([P, 2, NPAD], BF16, tag=f"hA{g}")
        nc.sync.dma_start(h0[:], hT0_d[:, :, g * NPAD:(g + 1) * NPAD])
        hA.append(h0)
        h1 = st.tile([P, 2, NPAD], BF16, tag=f"hB{g}", name=f"hB{g}")
        hB.append(h1)

    # ================= GGNN steps =================
    def emit_m(hg):
        """m = h @ Wcat (bf16 matmuls; fp8 node-pair-major evac)."""
        m8s = []
        for j in range(2):
            m8t = mp.tile([P, 2, TD], FP8, tag="m8")
            for ko in range(2):
                i = 2 * j + ko
                for nt in range(2):
                    pm = psm.tile([P, 512], FP32, tag="ps")
                    for kt in range(2):
                        nc.tensor.matmul(
                            pm[:],
                            lhsT=hg[:, kt, i * P:(i + 1) * P],
                            rhs=wcat[:, kt, nt * 512:(nt + 1) * 512],
                            start=(kt == 0), stop=(kt == 1),
                        )
                    dst = m8t[:, ko, nt * 512:(nt + 1) * 512]
                    if (i + nt) % 2 == 0:
                        nc.vector.tensor_copy(dst, pm[:])
                    else:
                        nc.scalar.activation(dst, pm[:], AF.Identity)
            m8s.append(m8t)
        return m8s

    for step in range(NUM_STEPS):
        hcur = hA if step % 2 == 0 else hB
        hnxt = hB if step % 2 == 0 else hA
        m8_all = {0: emit_m(hcur[0])}
        for g in range(GPC):
            hg = hcur[g]
            # --- aggregation: a^T = m_stack^T @ A^T_g (DoubleRow fp8) ---
            m8s = m8_all.pop(g)
            pas = [psm.tile([P, 512], FP32, tag="ps", name=f"pa{mt}")
                   for mt in range(2)]
            for c in range(8):
                j, t = c // 4, c % 4
                for mt in range(2):
                    nc.tensor.matmul(
                        pas[mt][:],
                        lhsT=m8s[j][:, :, t * D + mt * P: t * D + (mt + 1) * P],
                        rhs=at8s[g][:, 2 * t + j, :, :],
                        start=(c == 0), stop=(c == 7), perf_mode=DR,
                    )
            # overlap next graph's m-compute with this graph's GRU chain
            if g + 1 < GPC:
                m8_all[g + 1] = emit_m(hcur[g + 1])
            ag = agp.tile([P, 2, NPAD], BF16, tag="ag")
            if blcs:
                nc.vector.tensor_tensor(ag[:, 0, :], pas[0][:],
                                        blcs[g][:, 0, :], op=ALU.add)
                nc.vector.tensor_tensor(ag[:, 1, :], pas[1][:],
                                        blcs[g][:, 1, :], op=ALU.add)
            else:
                nc.vector.tensor_copy(ag[:, 0, :], pas[0][:])
                nc.scalar.activation(ag[:, 1, :], pas[1][:], AF.Identity)

            # --- GRU (bf16); tail elementwise fused across feature halves ---
            rp = rzp.tile([P, 2, 512], BF16, tag="rz", name="rp")
            zp2 = rzp.tile([P, 2, 512], BF16, tag="rz", name="zp2")
            for mt in range(4):
                pr = psm.tile([P, 512], FP32, tag="ps")
                for kt in range(2):
                    nc.tensor.matmul(
                        pr[:], lhsT=wih[:, kt, mt * P:(mt + 1) * P],
                        rhs=ag[:, kt, :], start=(kt == 0), stop=False)
                for kt in range(2):
                    nc.tensor.matmul(
                        pr[:], lhsT=whh[:, kt, mt * P:(mt + 1) * P],
                        rhs=hg[:, kt, :], start=False, stop=(kt == 1))
                dst = rp if mt < 2 else zp2
                nc.scalar.activation(dst[:, mt % 2, :], pr[:], AF.Sigmoid,
                                     bias=brz[:, mt:mt + 1])
            t1 = gp.tile([P, 2, 512], BF16, tag="t1")
            for mt in range(2):
                pi = psm.tile([P, 512], FP32, tag="ps")
                for kt in range(2):
                    nc.tensor.matmul(
                        pi[:], lhsT=wih[:, kt, 2 * D + mt * P: 2 * D + (mt + 1) * P],
                        rhs=ag[:, kt, :], start=(kt == 0), stop=(kt == 1))
                ph = psm.tile([P, 512], FP32, tag="ps")
                for kt in range(2):
                    nc.tensor.matmul(
                        ph[:], lhsT=whh[:, kt, 2 * D + mt * P: 2 * D + (mt + 1) * P],
                        rhs=hg[:, kt, :], start=(kt == 0), stop=(kt == 1))
                nc.vector.scalar_tensor_tensor(
                    t1[:, mt, :], ph[:], bhhn[:, mt:mt + 1], rp[:, mt, :],
                    op0=ALU.add, op1=ALU.mult)
                nc.vector.scalar_tensor_tensor(
                    t1[:, mt, :], pi[:], bihn[:, mt:mt + 1], t1[:, mt, :],
                    op0=ALU.add, op1=ALU.add)
            nsb = gp.tile([P, 2, 512], BF16, tag="nsb")
            nc.scalar.activation(nsb[:], t1[:], AF.Tanh)
            dsb = gp.tile([P, 2, 512], BF16, tag="dsb")
            nc.gpsimd.tensor_tensor(dsb[:], hg[:, :, :], nsb[:],
                                    op=ALU.subtract)
            nc.gpsimd.tensor_tensor(dsb[:], zp2[:], dsb[:],
                                    op=ALU.mult)
            nc.vector.tensor_tensor(hnxt[g][:, :, :], nsb[:], dsb[:],
                                    op=ALU.add)

    hfin = hA if NUM_STEPS % 2 == 0 else hB

    # ================= conv readout (bf16) =================
    res = cst.tile([1, GPC], FP32)
    for g in range(GPC):
        hg = hfin[g]
        fg = feats[g]
        # --- Y branch ---
        y1p = []
        for mt in range(2):
            pm = psm.tile([P, 512], FP32, tag="ps")
            first = True
            for k in range(3):
                for kt in range(2):
                    nc.tensor.matmul(
                        pm[:, :L1],
                        lhsT=w1[:, k * 2 + kt, mt * P:(mt + 1) * P],
                        rhs=hg[:, kt, k:k + L1],
                        start=first, stop=(k == 2 and kt == 1))
                    first = False
            y1 = cvp.tile([P, 512], BF16, tag="y1")
            if mt % 2 == 0:
                nc.vector.tensor_scalar(y1[:, :L1], pm[:, :L1],
                                        cb1[:, mt:mt + 1], 0.0,
                                        op0=ALU.add, op1=ALU.max)
            else:
                nc.scalar.activation(y1[:, :L1], pm[:, :L1], AF.Relu,
                                     bias=cb1[:, mt:mt + 1])
            yp = cvp.tile([P, LP1], BF16, tag="y1p")
            nc.vector.tensor_tensor(yp[:], y1[:, 0:505:2], y1[:, 1:506:2],
                                    op=ALU.max)
            nc.vector.tensor_tensor(yp[:], yp[:], y1[:, 2:507:2],
                                    op=ALU.max)
            y1p.append(yp)
        y2p = []
        for mt in range(2):
            pm = psm.tile([P, 512], FP32, tag="ps")
            for kt in range(2):
                nc.tensor.matmul(
                    pm[:, :L2Y],
                    lhsT=w2[:, kt, mt * P:(mt + 1) * P],
                    rhs=y1p[kt][:],
                    start=(kt == 0), stop=(kt == 1))
            y2 = cvp.tile([P, L2Y], BF16, tag="y2")
            if mt % 2 == 0:
                nc.vector.tensor_scalar(y2[:], pm[:, :L2Y],
                                        cb2[:, mt:mt + 1], 0.0,
                                        op0=ALU.add, op1=ALU.max)
            else:
                nc.scalar.activation(y2[:], pm[:, :L2Y], AF.Relu,
                                     bias=cb2[:, mt:mt + 1])
            yp = cvp.tile([P, LF], BF16, tag="y2p")
            nc.vector.tensor_tensor(yp[:], y2[:, 0:251:2], y2[:, 1:252:2],
                                    op=ALU.max)
            y2p.append(yp)
        pv = psm.tile([P, 512], FP32, tag="ps")
        for kt in range(2):
            nc.tensor.matmul(pv[0:1, :LF], lhsT=wy[:, kt, :],
                             rhs=y2p[kt][:], start=(kt == 0), stop=(kt == 1))
        ysb = cvp.tile([1, LF], FP32, tag="ysb")
        nc.scalar.activation(ysb[:], pv[0:1, :LF], AF.Identity,
                             bias=mlpb[:, 0:1])

        # --- Z branch (channels = [h; feat]) ---
        z1p = []
        for mt in range(4):
            pm = psm.tile([P, 512], FP32, tag="ps")
            first = True
            for k in range(3):
                for kt in range(4):
                    src = hg if kt < 2 else fg
                    nc.tensor.matmul(
                        pm[:, :L1],
                        lhsT=wc1[:, k * 4 + kt, mt * P:(mt + 1) * P],
                        rhs=src[:, kt % 2, k:k + L1],
                        start=first, stop=(k == 2 and kt == 3))
                    first = False
            z1 = cvp.tile([P, 512], BF16, tag="z1")
            if mt % 2 == 0:
                nc.vector.tensor_scalar(z1[:, :L1], pm[:, :L1],
                                        cc1[:, mt:mt + 1], 0.0,
                                        op0=ALU.add, op1=ALU.max)
            else:
                nc.scalar.activation(z1[:, :L1], pm[:, :L1], AF.Relu,
                                     bias=cc1[:, mt:mt + 1])
            zp = zpp.tile([P, LP1], BF16, tag="z1p")
            nc.vector.tensor_tensor(zp[:], z1[:, 0:505:2], z1[:, 1:506:2],
                                    op=ALU.max)
            nc.vector.tensor_tensor(zp[:], zp[:], z1[:, 2:507:2],
                                    op=ALU.max)
            z1p.append(zp)
        z2p = []
        for mt in range(4):
            pm = psm.tile([P, 512], FP32, tag="ps")
            first = True
            for k in range(2):
                for kt in range(4):
                    nc.tensor.matmul(
                        pm[:, :L2Z],
                        lhsT=wc2[:, k * 4 + kt, mt * P:(mt + 1) * P],
                        rhs=z1p[kt][:, k:k + L2Z],
                        start=first, stop=(k == 1 and kt == 3))
                    first = False
            z2 = cvp.tile([P, L2Z], BF16, tag="z2")
            if mt % 2 == 0:
                nc.vector.tensor_scalar(z2[:], pm[:, :L2Z],
                                        cc2[:, mt:mt + 1], 0.0,
                                        op0=ALU.add, op1=ALU.max)
            else:
                nc.scalar.activation(z2[:], pm[:, :L2Z], AF.Relu,
                                     bias=cc2[:, mt:mt + 1])
            zp = zpp.tile([P, LF], BF16, tag="z2p")
            nc.vector.tensor_tensor(zp[:], z2[:, 0:251:2], z2[:, 1:252:2],
                                    op=ALU.max)
            z2p.append(zp)
        pv = psm.tile([P, 512], FP32, tag="ps")
        for kt in range(4):
            nc.tensor.matmul(pv[0:1, :LF], lhsT=wz[:, kt, :],
                             rhs=z2p[kt][:], start=(kt == 0), stop=(kt == 3))
        zsb = cvp.tile([1, LF], FP32, tag="zsb")
        nc.scalar.activation(zsb[:], pv[0:1, :LF], AF.Identity,
                             bias=mlpb[:, 1:2])

        # --- final: sigmoid(mean(y*z)) ---
        prod = cvp.tile([1, LF], FP32, tag="prod")
        nc.vector.tensor_mul(prod[:], ysb[:], zsb[:])
        ssum = cvp.tile([1, 1], FP32, tag="ssum")
        nc.vector.reduce_sum(ssum[:], prod[:], axis=mybir.AxisListType.X)
        nc.scalar.activation(res[:, g:g + 1], ssum[:], AF.Sigmoid,
                             scale=1.0 / LF)

    for g in range(GPC):
        emit_conv(g)

    nc.sync.dma_start(out_d[:], res[:])


def _split_part(a, ntile):
    """[ntile*128, F...] -> [128, ntile, F...] with [p, t, ...] = a[t*128+p, ...]."""
    return np.ascontiguousarray(
        a.reshape(ntile, P, *a.shape[1:]).transpose(1, 0, *range(2, a.ndim + 1)))


def _prep_inputs(inputs):
    feat = np.asarray(inputs["feat"], np.float32)
    esrc = np.asarray(inputs["edge_src"]).astype(np.int64)
    edst = np.asarray(inputs["edge_dst"]).astype(np.int64)
    etyp = np.asarray(inputs["edge_type"]).astype(np.int64)

    # feature-major padded feat^T: per graph [256, 512]
    ftp = np.zeros((B, DIN, NPAD), np.float32)
    ftp[:, :, :N] = feat.transpose(0, 2, 1)

    # adjacency: AT[g, ki, t, j, ko, d] = #edges(type=t, src=256j+128ko+ki, dst=d)
    AT_all = np.zeros((B, P, T, 2, 2, NPAD), np.float32)
    DT_all = np.zeros((B, T, NPAD), np.float32)
    g_of = esrc // N
    s_l = esrc % N
    d_l = edst % N
    np.add.at(AT_all, (g_of, s_l % P, etyp, s_l // 256, (s_l % 256) // P, d_l), 1.0)
    np.add.at(DT_all, (g_of, etyp, d_l), 1.0)
    AT_all = AT_all.reshape(B, P, 8, 2, NPAD)
    b_lin = np.asarray(inputs["b_lin"], np.float32)
    use_blc = bool(np.any(b_lin))

    W_lin = np.asarray(inputs["W_lin"], np.float32)
    Wcat = W_lin.transpose(2, 0, 1).reshape(D, TD)
    W_ih = np.asarray(inputs["W_ih"], np.float32)
    W_hh = np.asarray(inputs["W_hh"], np.float32)
    b_ih = np.asarray(inputs["b_ih"], np.float32)
    b_hh = np.asarray(inputs["b_hh"], np.float32)

    def convT(w):  # [O, I, K] -> [128, K*ktiles, O]
        O, I, K = w.shape
        kt = I // P
        arr = w.transpose(2, 1, 0).reshape(K, kt, P, O).transpose(2, 0, 1, 3)
        return np.ascontiguousarray(arr.reshape(P, K * kt, O))

    common = {
        "Wcat": _split_part(Wcat, 2).astype(bf16),
        "WihT": _split_part(np.ascontiguousarray(W_ih.T), 2).astype(bf16),
        "WhhT": _split_part(np.ascontiguousarray(W_hh.T), 2).astype(bf16),
        "brz": np.ascontiguousarray((b_ih + b_hh)[:2 * D].reshape(4, P).T),
        "bihn": np.ascontiguousarray(b_ih[2 * D:].reshape(2, P).T),
        "bhhn": np.ascontiguousarray(b_hh[2 * D:].reshape(2, P).T),
        "W1T": convT(np.asarray(inputs["conv1_w"], np.float32)).astype(bf16),
        "W2T": convT(np.asarray(inputs["conv2_w"], np.float32)).astype(bf16),
        "Wc1T": convT(np.asarray(inputs["convc1_w"], np.float32)).astype(bf16),
        "Wc2T": convT(np.asarray(inputs["convc2_w"], np.float32)).astype(bf16),
        "cb1": np.ascontiguousarray(np.asarray(inputs["conv1_b"], np.float32).reshape(2, P).T),
        "cb2": np.ascontiguousarray(np.asarray(inputs["conv2_b"], np.float32).reshape(2, P).T),
        "cc1": np.ascontiguousarray(np.asarray(inputs["convc1_b"], np.float32).reshape(4, P).T),
        "cc2": np.ascontiguousarray(np.asarray(inputs["convc2_b"], np.float32).reshape(4, P).T),
        "wyT": _split_part(np.ascontiguousarray(np.asarray(inputs["mlp_y_w"], np.float32).T), 2).astype(bf16),
        "wzT": _split_part(np.ascontiguousarray(np.asarray(inputs["mlp_z_w"], np.float32).T), 4).astype(bf16),
        "mlpb": np.array([[float(np.asarray(inputs["mlp_y_b"])[0]),
                           float(np.asarray(inputs["mlp_z_b"])[0])]], np.float32),
    }

    in_maps = []
    for c in range(N_CORES):
        sl = slice(c * GPC, (c + 1) * GPC)
        hT0 = ftp[sl].transpose(1, 0, 2).reshape(DIN, GPC * NPAD)
        m = dict(common)
        m["hT0"] = _split_part(hT0, 2).astype(bf16)
        m["AT8"] = AT_all[sl].astype(f8)
        if use_blc:
            # blc[f, d] = sum_t b_lin[t, f] * indeg_t[d], per graph
            blc = np.einsum("tf,gtd->gfd", b_lin, DT_all[sl])
            blc = blc.transpose(1, 0, 2).reshape(D, GPC * NPAD)
            m["BLC"] = _split_part(blc, 2).astype(bf16)
        in_maps.append(m)
    return in_maps


def kernel(**inputs):
    use_blc = bool(np.any(np.asarray(inputs["b_lin"])))
    key = ("nc", use_blc)
    if key not in _NC_CACHE:
        _NC_CACHE[key] = _build_nc(use_blc=use_blc)
    nc = _NC_CACHE[key]
    in_maps = _prep_inputs(inputs)
    res = run_bass_kernel_spmd(nc, in_maps, list(range(N_CORES)))
    return np.concatenate([res.results[c]["out"][0] for c in range(N_CORES)])
